# revision 8
# baseline (speedup 1.0000x reference)
"""Trainium2 Bass kernel for nn_AqtConvBlock_12549894439421.

Computes relu(batchnorm(conv3x3_same(x, k), gamma, beta)) for
x [32,112,112,128] f32, k [3,3,128,256] f32 (NHWC / HWIO), with BN batch
statistics over (N,H,W).

The quantization scaling in the reference is pure scaling (no rounding or
clipping); conv is linear and BN normalizes any per-tensor scale away, so
y_ref == BN(conv(x,k)) up to an eps/c^2 perturbation ~2.5e-6 relative —
far below fp32 conv noise.

Sharding: data-parallel over batch (4 images per core, 8 cores).

Per core, channel-half-split pipeline (half = 128 of the 256 cout):
  conv(half0) -> allreduce stats0 -> [ conv(half1) || pass2(half0) ]
  -> allreduce stats1 -> pass2(half1)
so half0's normalize+relu+store hides under half1's conv.

conv: 3x3 conv as 9 shift-matmuls per output tile on the PE (cin=128 on
partitions, kernel slices stationary, 456-wide moving tiles over a
zero-padded 114-wide flattened image). Epilogue per tile: zero the 2
garbage columns in PSUM (memset), then one fused DVE tensor_scalar that
casts PSUM->bf16 y AND emits the per-channel sum, then one ACT Square
(reading the bf16 y) that emits the per-channel sum-of-squares. 77/112 of
y stays resident in SBUF; the rest spills to DRAM. BN stats (sum/sumsq per
channel) are all-reduced across cores on-chip (a tiny warmup AllReduce at
t=0 hides collective init).

Host side does layout marshalling only: pad/transpose/cast x to a
cin-major zero-padded image layout, pack weights, strip the pad columns
and reassemble NHWC output from the per-core channel-major results.
"""

import numpy as np
import ml_dtypes

import concourse.bacc as bacc
import concourse.tile as tile
import concourse.mybir as mybir
from concourse import bass_utils

F32 = mybir.dt.float32
BF16 = mybir.dt.bfloat16
AF = mybir.ActivationFunctionType
ALU = mybir.AluOpType
AX = mybir.AxisListType

N_CORES = 8
N, H, W, CIN, COUT = 32, 112, 112, 128, 256
NP = N // N_CORES          # images per core
HP, WP = H + 3, W + 2      # padded image incl. 1px halo + 1 extra zero row
IMG = HP * WP              # 13110 flat padded pixels per image
GW = W + 2                 # padded output row width (2 garbage cols)
G = H * GW                 # 12768 flat padded output pixels per image
RPT = 4                    # output rows per matmul tile
TW = RPT * GW              # 456 moving free dim per matmul
NT = G // TW               # 28 tiles per image
NQ = 7                     # x-load quads per image (4 tiles each)
QT = 4
XC = QT * TW + 2 * GW + 2  # 2054 x elems per quad load (incl. halo)
GCOLS = NP * NT            # 112 tiles per half
RT = 70                    # resident tiles per half (rest spill to DRAM)
SPT = GCOLS - RT           # 35 spilled tiles
NPIXP = NP * G             # 51072 padded out pixels per core (per half)
NTOT = N * H * W           # BN statistics count
BN_EPS = 1e-5
P2C = 1596                 # pass-2 chunk; RT*456 = 22*P2C, SPT*456 = 10*P2C
RES_CH = RT * TW // P2C    # 22
SP_CH = SPT * TW // P2C    # 10

_CACHE = {}


def _build():
    nc = bacc.Bacc("TRN2", target_bir_lowering=False, debug=False,
                   num_devices=N_CORES)
    x_d = nc.dram_tensor("x", [128, NP * IMG], BF16, kind="ExternalInput").ap()
    w_d = nc.dram_tensor("w", [128, 2 * 9 * 128], BF16, kind="ExternalInput").ap()
    gb_d = nc.dram_tensor("gb", [128, 4], F32, kind="ExternalInput").ap()
    out_d = nc.dram_tensor("out", [2, 128, NPIXP], F32, kind="ExternalOutput").ap()

    with tile.TileContext(nc) as tc:
        with tc.tile_pool(name="const", bufs=1) as cp, \
             tc.tile_pool(name="xin", bufs=3) as xp, \
             tc.tile_pool(name="ysb", bufs=16) as yp, \
             tc.tile_pool(name="sq", bufs=2) as sqp, \
             tc.tile_pool(name="stats", bufs=1) as stp, \
             tc.tile_pool(name="p2i", bufs=2) as p2i, \
             tc.tile_pool(name="p2o", bufs=3) as p2o, \
             tc.tile_pool(name="ps", bufs=1, space="PSUM") as pp, \
             tc.tile_pool(name="dram", bufs=1, space="DRAM") as dp:

            # collective warmup: tiny AllReduce with no deps, runs at t=0
            ccw_i = dp.tile([128, 2], F32, name="ccw_i", tag="ccw_i")
            ccw_o = dp.tile([128, 2], F32, name="ccw_o", tag="ccw_o")
            nc.gpsimd.collective_compute(
                "AllReduce", ALU.add,
                replica_groups=[list(range(N_CORES))],
                ins=[ccw_i.opt()], outs=[ccw_o.opt()])

            w_sb = cp.tile([128, 2 * 9 * 128], BF16)
            nc.sync.dma_start(w_sb[:], w_d[:])
            gb_sb = cp.tile([128, 4], F32)
            nc.sync.dma_start(gb_sb[:], gb_d[:])

            y_res = [stp.tile([128, RT * TW], BF16, name=f"yres{h}",
                              tag=f"yres{h}") for h in range(2)]
            y_d = [dp.tile([128, SPT * TW], BF16, name=f"yd{h}", tag=f"yd{h}")
                   for h in range(2)]
            sums = [stp.tile([128, GCOLS], F32, name=f"sum{h}", tag=f"sum{h}")
                    for h in range(2)]
            ssqs = [stp.tile([128, GCOLS], F32, name=f"ssq{h}", tag=f"ssq{h}")
                    for h in range(2)]
            stat2 = [stp.tile([128, 2], F32, name=f"st2_{h}", tag=f"st2_{h}")
                     for h in range(2)]
            red = [stp.tile([128, 2], F32, name=f"red{h}", tag=f"red{h}")
                   for h in range(2)]
            ab = [stp.tile([128, 2], F32, name=f"ab{h}", tag=f"ab{h}")
                  for h in range(2)]
            tmp = stp.tile([128, 8], F32)
            cc_i = [dp.tile([128, 2], F32, name=f"cci{h}", tag=f"cci{h}")
                    for h in range(2)]
            cc_o = [dp.tile([128, 2], F32, name=f"cco{h}", tag=f"cco{h}")
                    for h in range(2)]

            def conv_quad(half, img, q):
                xc = xp.tile([128, XC], BF16)
                nc.sync.dma_start(
                    xc[:], x_d[:, img * IMG + q * QT * TW:
                               img * IMG + q * QT * TW + XC])
                for ti in range(QT):
                    t = q * QT + ti
                    gcol = img * NT + t
                    ps = pp.tile([128, TW], F32, bufs=6)
                    for p in range(9):
                        kh, kw = p // 3, p % 3
                        blk = (half * 9 + p) * 128
                        off = ti * TW + kh * GW + kw
                        nc.tensor.matmul(ps[:], w_sb[:, blk:blk + 128],
                                         xc[:, off:off + TW],
                                         start=(p == 0), stop=(p == 8))
                    garb = ps[:].rearrange("p (r w) -> p r w", r=RPT)[:, :, W:GW]
                    nc.vector.memset(garb, 0.0)
                    if gcol < RT:
                        y_dest = y_res[half][:, gcol * TW:(gcol + 1) * TW]
                    else:
                        y_sb = yp.tile([128, TW], BF16)
                        y_dest = y_sb[:]
                    nc.vector.tensor_scalar(
                        y_dest, ps[:], 1.0, None, op0=ALU.mult, op1=ALU.add,
                        accum_out=sums[half][:, gcol:gcol + 1])
                    sq = sqp.tile([128, TW], F32)
                    nc.scalar.activation(
                        sq[:], y_dest, AF.Square,
                        accum_out=ssqs[half][:, gcol:gcol + 1])
                    if gcol >= RT:
                        nc.sync.dma_start(
                            y_d[half][:, (gcol - RT) * TW:(gcol - RT + 1) * TW],
                            y_dest)

            def stats_reduce_and_cc(half):
                nc.vector.reduce_sum(stat2[half][:, 0:1], sums[half][:], axis=AX.X)
                nc.vector.reduce_sum(stat2[half][:, 1:2], ssqs[half][:], axis=AX.X)
                nc.sync.dma_start(cc_i[half][:], stat2[half][:])
                nc.gpsimd.collective_compute(
                    "AllReduce", ALU.add,
                    replica_groups=[list(range(N_CORES))],
                    ins=[cc_i[half].opt()], outs=[cc_o[half].opt()])
                nc.sync.dma_start(red[half][:], cc_o[half][:])

            def stats_math(half):
                # a = gamma * rsqrt(var+eps); b = beta - mean*a
                h = half
                mean = tmp[:, 4 * h + 0:4 * h + 1]
                var = tmp[:, 4 * h + 1:4 * h + 2]
                std = tmp[:, 4 * h + 2:4 * h + 3]
                rstd = tmp[:, 4 * h + 3:4 * h + 4]
                a = ab[h][:, 0:1]
                b = ab[h][:, 1:2]
                inv_n = 1.0 / float(NTOT)
                nc.vector.tensor_scalar_mul(mean, red[h][:, 0:1], inv_n)
                nc.vector.tensor_scalar_mul(var, red[h][:, 1:2], inv_n)
                nc.vector.tensor_tensor(std, mean, mean, op=ALU.mult)
                nc.vector.tensor_tensor(var, var, std, op=ALU.subtract)
                nc.vector.tensor_scalar_add(var, var, BN_EPS)
                nc.scalar.activation(std, var, AF.Sqrt)
                nc.vector.reciprocal(rstd, std)
                nc.vector.tensor_tensor(a, gb_sb[:, 2 * h:2 * h + 1], rstd,
                                        op=ALU.mult)
                nc.vector.tensor_tensor(b, mean, a, op=ALU.mult)
                nc.vector.tensor_tensor(b, gb_sb[:, 2 * h + 1:2 * h + 2], b,
                                        op=ALU.subtract)

            def pass2_chunk(half, c, prefetched=None):
                a = ab[half][:, 0:1]
                b = ab[half][:, 1:2]
                if c < RES_CH:
                    src = y_res[half][:, c * P2C:(c + 1) * P2C]
                else:
                    cs = c - RES_CH
                    if prefetched and c in prefetched:
                        src = prefetched[c][:]
                    else:
                        yt = p2i.tile([128, P2C], BF16)
                        nc.scalar.dma_start(
                            yt[:], y_d[half][:, cs * P2C:(cs + 1) * P2C])
                        src = yt[:]
                ot = p2o.tile([128, P2C], F32)
                nc.scalar.activation(ot[:], src, AF.Relu, bias=b, scale=a)
                off = c * P2C
                nc.scalar.dma_start(out_d[half, :, off:off + P2C], ot[:])

            # ---- phase 0: conv half 0 ----
            for img in range(NP):
                for q in range(NQ):
                    conv_quad(0, img, q)
            stats_reduce_and_cc(0)
            # ---- phase 1: conv half 1, with half-0 pass 2 overlapped ----
            # Emit 4 quads of conv first so the CC-dependent stats math
            # doesn't head-block the DVE stream; then spread pass2(0) chunk
            # emission across the remaining quads so the in-order ACT/DMA
            # streams never build a backlog that starves the conv epilogue.
            half1_quads = [(0, q) for q in range(4, NQ)] + \
                [(img, q) for img in range(1, NP) for q in range(NQ)]
            for q in range(4):
                conv_quad(1, 0, q)
            stats_math(0)
            nchunks = RES_CH + SP_CH
            done = 0
            for i, (img, q) in enumerate(half1_quads):
                conv_quad(1, img, q)
                want = min(nchunks, (i + 1) * 3 // 2)
                while done < want:
                    pass2_chunk(0, done)
                    done += 1
            while done < nchunks:
                pass2_chunk(0, done)
                done += 1
            stats_reduce_and_cc(1)
            # prefetch some half-1 spill chunks while the collective runs
            pre = {}
            for c in range(RES_CH, RES_CH + 2):
                yt = p2i.tile([128, P2C], BF16, name=f"p2pre{c}",
                              tag=f"p2pre{c}", bufs=1)
                nc.scalar.dma_start(
                    yt[:], y_d[1][:, (c - RES_CH) * P2C:(c - RES_CH + 1) * P2C])
                pre[c] = yt
            stats_math(1)
            for c in range(nchunks):
                pass2_chunk(1, c, prefetched=pre)

    nc.compile()
    return nc


def _get_nc():
    if "nc" not in _CACHE:
        _CACHE["nc"] = _build()
    return _CACHE["nc"]


def _prep_inputs(x, kern, gamma, beta):
    xbf = x.astype(ml_dtypes.bfloat16)
    kbf = kern.astype(ml_dtypes.bfloat16)
    w_host = np.zeros((128, 2 * 9 * 128), dtype=ml_dtypes.bfloat16)
    for h in range(2):
        for p in range(9):
            kh, kw = p // 3, p % 3
            blk = (h * 9 + p) * 128
            w_host[:, blk:blk + 128] = kbf[kh, kw, :, h * 128:(h + 1) * 128]
    gb_host = np.stack([gamma[:128], beta[:128], gamma[128:], beta[128:]],
                       axis=1).astype(np.float32)
    gb_host = np.ascontiguousarray(gb_host)
    in_maps = []
    for c in range(N_CORES):
        xs = xbf[c * NP:(c + 1) * NP]                # [NP,112,112,128]
        xp_ = np.zeros((128, NP, HP, WP), dtype=ml_dtypes.bfloat16)
        xp_[:, :, 1:H + 1, 1:W + 1] = xs.transpose(3, 0, 1, 2)
        in_maps.append({"x": xp_.reshape(128, NP * IMG),
                        "w": w_host, "gb": gb_host})
    return in_maps


def _assemble(results):
    out = np.empty((N, H, W, COUT), dtype=np.float32)
    for c in range(N_CORES):
        o = results[c]["out"]                        # [2,128,NPIXP]
        oo = o.reshape(2, 128, NP, H, GW)[:, :, :, :, :W]
        out[c * NP:(c + 1) * NP] = oo.transpose(2, 3, 4, 0, 1).reshape(
            NP, H, W, COUT)
    return out


def _run(in_maps, trace=False, **kw):
    nc = _get_nc()
    return bass_utils.run_bass_kernel_spmd(
        nc, in_maps, core_ids=list(range(N_CORES)), trace=trace, **kw)


def kernel(x, kernel, gamma, beta):
    in_maps = _prep_inputs(x, kernel, gamma, beta)
    res = _run(in_maps)
    return _assemble(res.results)
